# revision 39
# baseline (speedup 1.0000x reference)
"""Multi-head attention (B=4, S=2048, E=1024, H=16, D=64) on 8 trn2 cores.

Sharding: core c handles batch b=c//2 and head-group hg=c%2 (8 heads, 512
embed cols). QKV projection weights are column-sharded by head group so
attention is fully local per device.

Per-core plan (bf16 matmul operands, fp32 PSUM accumulation):
  - The ACT (scalar) engine's exp stream is the roofline: 256 activations
    of [128, 1024] at ~1.11us each (~285us busy). The schedule exists to
    start that stream early and keep it fed; measured steady state runs
    at the ACT roofline (PE ~88% busy under it).
  - All casting loads (X s-tiles, W m-slices) ride the gpsimd cast-DMA
    queue, ordered so qk_proj(0,0)'s inputs land first. X is transposed
    into xt[e][:, s] with REGULAR matmuls against a bf16 identity (not
    transpose-mode) so the PE HAM clock-gate warms up from ~9us.
  - pair 0's four q-chunk windows run k-BLOCKED: in phase kb, window qc
    processes the k-tiles unlocked by X-chunk kb while X-chunk kb+1
    loads/transposes/projects in the PE gaps. Suspended windows park
    their ctx PSUM accumulator in SBUF (DVE copy / in-place add) and
    resume next phase, so only one window's ctx pair is PSUM-live.
  - pairs 1..3 run plain full-k windows with the next pair's Q/K
    projections spread between attention items.
  - scores^T[k,q] = KT.T @ QT per head-pair: two K=64 matmuls row-tiled
    at partition bases 0/64 (concurrent in the PE array).
  - ctx^T_aug[65,q] += [V|1].T @ exp ; row 64 = softmax denominator Z.
    ctx matmuls trail the exp stream by 2 items (fully deferred in phase
    0 where V isn't projected yet); segment-end flush/suspend/finalize
    ops are delayed past the first 2 items of the NEXT segment so the PE
    queue never blocks on the last exp of a window.
  - output: DVE 32x32 block-transpose + per-q 1/Z scale (Z columns via a
    tiny DRAM bounce; gather on the gpsimd queue) + block-permuted DMA.
  - PSUM budget (8 banks): scores 2x[128,1024]f32 (4) + ctx 2x[65,512]f32
    (2) + shared proj/transpose pool 2x[128,512]f32 (2).
"""

import numpy as np
from contextlib import ExitStack

import concourse.bass as bass
import concourse.mybir as mybir
import concourse.tile as tile
from concourse.bass import ts, ds
from concourse.masks import make_identity

B, S, E = 4, 2048, 1024
H, DH = 16, 64
NCORES = 8
HG = 2                # head groups per batch (cores per batch element)
HPC = H // HG         # heads per core = 8
CE = HPC * DH         # embed cols per core = 512
P = 128
NQT = S // P          # 16 s-tiles of 128
QC = 4                # q chunks of 512
ET = E // P           # 8 e-tiles
MT = CE // P          # 4 output dim tiles (head pairs)

F32 = mybir.dt.float32
BF16 = mybir.dt.bfloat16
AF = mybir.ActivationFunctionType
ADD = mybir.AluOpType.add


def _build(tc, out, hs, wq, bq, wk, bk, wv, bv):
    nc = tc.nc
    with ExitStack() as ctx:
        persist = ctx.enter_context(tc.tile_pool(name="persist", bufs=1))
        xsp = ctx.enter_context(tc.tile_pool(name="xs_pool", bufs=4))
        ep = ctx.enter_context(tc.tile_pool(name="e_pool", bufs=8))
        cp = ctx.enter_context(tc.tile_pool(name="c_pool", bufs=4))
        otp = ctx.enter_context(tc.tile_pool(name="ot_pool", bufs=3))
        zp = ctx.enter_context(tc.tile_pool(name="z_pool", bufs=2))
        pp = ctx.enter_context(tc.tile_pool(name="part_pool", bufs=1))
        drp = ctx.enter_context(tc.tile_pool(name="dram_pool", bufs=2, space="DRAM"))
        # PSUM: proj/transpose shared pool (2 banks) + scores (4) + ctx (2)
        pjp = ctx.enter_context(tc.tile_pool(name="proj_psum", bufs=2, space="PSUM"))
        spp = ctx.enter_context(tc.tile_pool(name="s_psum", bufs=2, space="PSUM"))
        cpp = ctx.enter_context(tc.tile_pool(name="ctx_psum", bufs=2, space="PSUM"))

        # ---- persistent buffers ----
        qt = [persist.tile([P, S], BF16, tag=f"qt{m}", name=f"qt{m}")
              for m in range(MT)]
        kt = [persist.tile([P, S], BF16, tag=f"kt{m}", name=f"kt{m}")
              for m in range(MT)]
        v = [persist.tile([P, HPC, DH + 1], BF16, tag=f"v{st}", name=f"v{st}")
             for st in range(NQT)]
        xt = [persist.tile([P, S], BF16, tag=f"xt{e}", name=f"xt{e}")
              for e in range(ET)]
        bqs = persist.tile([P, MT], F32, tag="bqs")
        bks = persist.tile([P, MT], F32, tag="bks")
        bvrow = persist.tile([1, CE], BF16, tag="bvrow")
        ones_row = persist.tile([1, P], BF16, tag="ones_row")
        ones_col = persist.tile([P, HPC], BF16, tag="ones_col")
        ident = persist.tile([P, P], BF16, tag="ident")
        ws = {}
        for nm in ("wq", "wk", "wv"):
            ws[nm] = persist.tile([P, ET, CE], BF16, tag=nm, name=nm)

        wqr = wq.rearrange("(o p) c -> p o c", p=P)
        wkr = wk.rearrange("(o p) c -> p o c", p=P)
        wvr = wv.rearrange("(o p) c -> p o c", p=P)
        hsr = hs.rearrange("(t p) e -> p t e", p=P)  # [128, 16, 1024]

        # ---- building blocks ----
        def load_w_slice(nm, wsrc, m):
            nc.gpsimd.dma_start(ws[nm][:, :, ts(m, P)], wsrc[:, :, ts(m, P)])

        def load_x_dma(s):
            """gpsimd cast-DMA (fp32->bf16). The first tiles go as
            half-tiles so the first transposes unblock sooner."""
            xs_t = xsp.tile([P, E], BF16, tag="xs", name="xs")
            if s < 2:
                nc.gpsimd.dma_start(xs_t[:, 0:512], hsr[:, s, 0:512])
                nc.gpsimd.dma_start(xs_t[:, 512:1024], hsr[:, s, 512:1024])
            else:
                nc.gpsimd.dma_start(xs_t, hsr[:, s, :])
            return xs_t

        def warm_mm(n=1):
            """Full-array (K=128) matmuls into a never-read scratch tile.
            The HAM clock gate tracks real PE activity: K=1 dummies don't
            register, and transpose-mode doesn't either. A burst during
            the initial DMA wait unthrottles the PE to 2.4GHz; later
            singles keep the MID window from seeing a fully-idle 3.4us."""
            scratch = spp.tile([P, 1024], F32, tag="sps", name="warm")
            for _ in range(n):
                nc.tensor.matmul(scratch[:, 0:128], lhsT=ident,
                                 rhs=ident, start=True, stop=True)

        def transpose_half(xs_t, s, eg):
            # regular matmul vs identity (counts as PE-busy for HAM, unlike
            # transpose-mode); fp32 PSUM out, cast on the DVE copy
            tp = pjp.tile([P, 4, P], F32, tag="pps", name="tps")
            for j in range(4):
                nc.tensor.matmul(tp[:, j, :],
                                 lhsT=xs_t[:, ts(eg * 4 + j, P)],
                                 rhs=ident, start=True, stop=True)
            for j in range(4):
                nc.vector.tensor_copy(out=xt[eg * 4 + j][:, ts(s, P)],
                                      in_=tp[:, j, :])

        def v_proj_half(st, half, state):
            """V projection for s-tile st, e-tiles [4*half, 4*half+4)."""
            if half == 0:
                state["ps"] = pjp.tile([P, 512], F32, tag="pps", name="pps")
            ps = state["ps"]
            for e in range(4 * half, 4 * half + 4):
                nc.tensor.matmul(ps, lhsT=xt[e][:, ts(st, P)],
                                 rhs=ws["wv"][:, e, :],
                                 start=(e == 0), stop=False)
            if half == 1:
                nc.tensor.matmul(ps, lhsT=ones_row, rhs=bvrow,
                                 start=False, stop=True)
                nc.vector.tensor_copy(
                    out=v[st][:, :, 0:DH],
                    in_=ps.rearrange("p (h d) -> p h d", h=HPC),
                )
                nc.vector.tensor_copy(out=v[st][:, :, DH], in_=ones_col)

        def v_proj(st):
            state = {}
            v_proj_half(st, 0, state)
            v_proj_half(st, 1, state)

        def qk_proj_gen(m, qc):
            """Q/K projections for dim-tile m, q-chunk qc; yields per 4 e-MMs."""
            for wname, dstt, bias in (("wq", qt, bqs), ("wk", kt, bks)):
                ps = pjp.tile([P, 512], F32, tag="pps", name="pps")
                for e in range(ET):
                    nc.tensor.matmul(
                        ps,
                        lhsT=ws[wname][:, e, ts(m, P)],
                        rhs=xt[e][:, ts(qc, 512)],
                        start=(e == 0),
                        stop=(e == ET - 1),
                    )
                    if e % 4 == 3:
                        yield
                nc.vector.tensor_scalar_add(
                    dstt[m][:, ts(qc, 512)], ps, bias[:, ts(m, 1)]
                )
            yield

        def qk_proj(m, qc):
            for _ in qk_proj_gen(m, qc):
                pass

        # ---- attention window (one (pair, q-chunk)) ----
        class Seg:
            """One PSUM-residency segment of a window's ctx accumulation."""
            __slots__ = ("pends", "ctxA", "ctxB", "first")

            def __init__(self):
                self.pends = []
                self.ctxA = self.ctxB = None
                self.first = False

        class Win:
            def __init__(self, pr, qc):
                self.pr, self.qc = pr, qc
                self.hA, self.hB = 2 * pr, 2 * pr + 1
                self.seg = Seg()
                self.stagger = 2    # None = defer all until flush
                self.pA = self.pB = None

            def _ctx_mm(self, seg, pk, pe, stop):
                if seg.ctxA is None:
                    # lazy open: ensures the previous segment's suspend
                    # (possibly emitted 2 items into THIS segment) precedes
                    # the slot reuse in emission order
                    seg.ctxA = cpp.tile([DH + 1, 512], F32, tag="ctx", name="ctx")
                    seg.ctxB = cpp.tile([DH + 1, 512], F32, tag="ctx", name="ctx")
                    seg.first = True
                nc.tensor.matmul(seg.ctxA, lhsT=v[pk][:, self.hA, :],
                                 rhs=pe[:, 0:512],
                                 start=seg.first, stop=stop)
                nc.tensor.matmul(seg.ctxB, lhsT=v[pk][:, self.hB, :],
                                 rhs=pe[:, 512:1024],
                                 start=seg.first, stop=stop)
                seg.first = False

            def item(self, kti):
                sps = spp.tile([P, 1024], F32, tag="sps", name="sps")
                nc.tensor.matmul(
                    sps[:, 0:512],
                    lhsT=kt[self.pr][0:DH, ts(kti, P)],
                    rhs=qt[self.pr][0:DH, ts(self.qc, 512)],
                    start=True, stop=True,
                )
                nc.tensor.matmul(
                    sps[:, 512:1024],
                    lhsT=kt[self.pr][DH:P, ts(kti, P)],
                    rhs=qt[self.pr][DH:P, ts(self.qc, 512)],
                    start=True, stop=True,
                )
                et = ep.tile([P, 1024], BF16, tag="expT", name="expT")
                nc.scalar.activation(et, sps, AF.Exp, scale=0.125)
                self.seg.pends.append((kti, et))
                if self.stagger is not None and len(self.seg.pends) > self.stagger:
                    self._ctx_mm(self.seg, *self.seg.pends.pop(0), stop=False)

            def _flush(self, seg):
                while seg.pends:
                    pk, pe = seg.pends.pop(0)
                    self._ctx_mm(seg, pk, pe, stop=(not seg.pends))

            def suspend(self):
                """Returns a closure that parks the CURRENT segment in SBUF;
                the window immediately starts a fresh segment so later items
                don't disturb the captured one."""
                seg, self.seg = self.seg, Seg()

                def emit():
                    self._flush(seg)
                    parts = []
                    for ctx_t, part in ((seg.ctxA, self.pA), (seg.ctxB, self.pB)):
                        if part is None:
                            part = pp.tile([DH + 1, 512], F32,
                                           tag=f"part_q{self.qc}_{len(parts)}",
                                           name="part")
                            nc.vector.tensor_copy(out=part, in_=ctx_t)
                        else:
                            nc.vector.tensor_tensor(part, ctx_t, part, ADD)
                        parts.append(part)
                    self.pA, self.pB = parts

                return emit

            def finalize(self):
                """Returns a closure emitting the normalize+store pipeline
                for the captured final segment."""
                seg, self.seg = self.seg, Seg()

                def emit():
                    self._finalize_emit(seg)

                return emit

            def _finalize_emit(self, seg):
                self._flush(seg)
                zd = drp.tile([2, 2, 512], F32, tag="zd", name="zd")
                css = []
                for idx, (ctx_t, part) in enumerate(
                        ((seg.ctxA, self.pA), (seg.ctxB, self.pB))):
                    cs = cp.tile([DH + 1, 512], F32, tag="cs", name="cs")
                    if part is None:
                        nc.vector.tensor_copy(out=cs, in_=ctx_t)
                    else:
                        nc.vector.tensor_tensor(cs, ctx_t, part, ADD)
                    # bounce raw Z through DRAM (twice: avoids step-0 DRAM AP)
                    nc.sync.dma_start(zd[0, idx][None, :], cs[DH : DH + 1, :])
                    nc.sync.dma_start(zd[1, idx][None, :], cs[DH : DH + 1, :])
                    css.append(cs)
                self.ctxA = self.ctxB = None
                # fold Z rows into per-q columns matching the 32x32
                # block-transposed layout, then reciprocal over 64 lanes
                c2 = zp.tile([DH, 2, NQT], F32, tag="c2", name="c2")
                for i in range(2):
                    nc.gpsimd.dma_start(
                        c2[ts(i, 32)],
                        zd[i].rearrange("h (j a) -> a h j", a=32),
                    )
                nc.vector.reciprocal(c2, c2)
                for idx, hl in ((0, self.hA), (1, self.hB)):
                    bt = otp.tile([DH, 512], F32, tag="bt", name="bt")
                    nc.vector.transpose(bt, css[idx][0:DH, :])
                    ot = otp.tile([DH, NQT, 32], F32, tag="ot", name="ot")
                    nc.vector.tensor_tensor(
                        ot,
                        bt.rearrange("p (j b) -> p j b", b=32),
                        c2[:, idx, :, None].to_broadcast([DH, NQT, 32]),
                        mybir.AluOpType.mult,
                    )
                    # block-permuted store: ot[32i+a, j, b] -> row qc*512+32j+a,
                    # col hl*64+32i+b
                    for i in range(2):
                        nc.gpsimd.dma_start(
                            out.rearrange(
                                "(qq j a) (h i b) -> qq h i a j b",
                                j=NQT, a=32, i=2, b=32,
                            )[self.qc, hl, i],
                            ot[ts(i, 32)],
                        )

        # ---- interleaved phase driver with cross-segment end delay ----
        def run_phase(events, preps, carry_in=None, hold_last=True,
                      carry_delay=2, prep_frac=1.0):
            """events: list of ("item", fn) | ("end", factory). At the end
            event's stream position the factory runs (capturing the live
            segment); its emit closure is delayed past the next 2 items so
            the next segment's scores hide the last-exp wait. The final
            emit can carry into the next phase. preps are spread between
            emissions."""
            fns = []
            helds = [[carry_in, carry_delay]] if carry_in is not None else []
            for kind, fn in events:
                if kind == "item":
                    fns.append(fn)
                    for h in helds:
                        h[1] -= 1
                    while helds and helds[0][1] <= 0:
                        fns.append(helds.pop(0)[0])
                else:
                    cell = [None, fn]
                    fns.append(lambda cell=cell: cell.__setitem__(0, cell[1]()))
                    helds.append([lambda cell=cell: cell[0](), 2])
            carry_out = None
            if helds:
                if hold_last:
                    *rest, last = helds
                    fns.extend(h[0] for h in rest)
                    carry_out = last[0]
                else:
                    fns.extend(h[0] for h in helds)
            n_i, n_p = len(fns), len(preps)
            pi = 0
            for i, fn in enumerate(fns):
                fn()
                # prep_frac < 1 front-loads the preps so their trailing DVE
                # (the qk bias) clears before the window-boundary DVE burst
                while pi < n_p and pi * n_i * prep_frac < (i + 1) * n_p:
                    preps[pi]()
                    pi += 1
            while pi < n_p:
                preps[pi]()
                pi += 1
            return carry_out

        def x_chunk_preps(kb, extra_dmas=()):
            """Prep closures for X chunk kb (s-tiles 4kb..4kb+3) + qk(0,kb).
            X DMAs+transposes with the extra (W) DMAs spread between the X
            tiles on the gpsimd queue. V projections are NOT here — they
            ride the next phase's event stream in readiness order."""
            preps = []
            extra_dmas = list(extra_dmas)
            n_ed = len(extra_dmas)
            for si, s in enumerate(range(4 * kb, 4 * kb + 4)):
                box = {}

                def dma(s=s, box=box):
                    box["xs"] = load_x_dma(s)

                preps.append(dma)
                for eg in range(2):
                    preps.append(lambda s=s, eg=eg, box=box:
                                 transpose_half(box["xs"], s, eg))
                # spread the extra W DMAs between the X tiles on the queue
                preps += extra_dmas[si * n_ed // 4:(si + 1) * n_ed // 4]
            g = qk_proj_gen(0, kb)
            preps += [lambda g=g: next(g, None)] * 5
            return preps

        # ---- emission ----
        # prologue. The identity/memsets go first (they occupy the gpsimd
        # engine queue which also dispatches the casting DMAs). Then the
        # two DMA queues race in parallel: gpsimd carries X s0 + all W
        # slices (cast-DMA is gpsimd-only), sync carries fp32 X s1..s3.
        nc.vector.memset(ones_row, 1.0)
        nc.vector.memset(ones_col, 1.0)
        make_identity(nc, ident)
        xs03 = [load_x_dma(s) for s in range(4)]
        load_w_slice("wq", wqr, 0)
        load_w_slice("wk", wkr, 0)
        nc.sync.dma_start(bqs, bq.rearrange("(o p) -> p o", p=P))
        nc.sync.dma_start(bks, bk.rearrange("(o p) -> p o", p=P))
        # warm the HAM clock gate during the DMA wait (PE is idle anyway);
        # 40 MMs ~= 4.3us guarantees one fully-busy free-running window
        warm_mm(40)
        for s in range(4):
            for eg in range(2):
                transpose_half(xs03[s], s, eg)
        qk_proj(0, 0)

        # pair-0 phases: phase kb runs the k-tiles unlocked by chunk kb for
        # windows 0..kb while chunk kb+1 (or pair-1 projections) preps.
        # V projections for chunk c are EVENTS of phase c+1 (where Wv/X have
        # certainly landed), positioned just before their ctx consumers so
        # the in-order PE queue matches runtime readiness.
        wins0 = [Win(0, qc) for qc in range(QC)]
        wins0[0].stagger = None    # phase-0 ctx fully deferred (V not ready)

        def it(w, kti):
            return ("item", lambda w=w, kti=kti: w.item(kti))

        def vp(st):
            return ("item", lambda st=st: v_proj(st))

        w0, w1, w2, w3 = wins0
        phase_events = [
            # phase 0: only chunk-0 scores; ctx deferred into the carried
            # suspend which lands in phase 1 after v0..3
            [it(w0, 0), it(w0, 1), it(w0, 2), it(w0, 3),
             ("end", w0.suspend)],
            # phase 1 (carry_delay=6: held ph0-suspend emits after v3)
            [it(w0, 4), it(w0, 5), vp(0), vp(1), vp(2), vp(3),
             vp(4), it(w0, 6), vp(5), it(w0, 7), vp(6), vp(7),
             ("end", w0.suspend),
             it(w1, 0), it(w1, 1), it(w1, 2), it(w1, 3),
             it(w1, 4), it(w1, 5), it(w1, 6), it(w1, 7),
             ("end", w1.suspend)],
            # phase 2
            [it(w0, 8), it(w0, 9), vp(8), it(w0, 10), vp(9), it(w0, 11),
             vp(10), vp(11), ("end", w0.suspend),
             it(w1, 8), it(w1, 9), it(w1, 10), it(w1, 11),
             ("end", w1.suspend),
             it(w2, 0), it(w2, 1), it(w2, 2), it(w2, 3),
             it(w2, 4), it(w2, 5), it(w2, 6), it(w2, 7),
             it(w2, 8), it(w2, 9), it(w2, 10), it(w2, 11),
             ("end", w2.suspend)],
            # phase 3
            [it(w0, 12), it(w0, 13), vp(12), it(w0, 14), vp(13),
             it(w0, 15), vp(14), vp(15), ("end", w0.finalize),
             it(w1, 12), it(w1, 13), it(w1, 14), it(w1, 15),
             ("end", w1.finalize),
             it(w2, 12), it(w2, 13), it(w2, 14), it(w2, 15),
             ("end", w2.finalize),
             it(w3, 0), it(w3, 1), it(w3, 2), it(w3, 3),
             it(w3, 4), it(w3, 5), it(w3, 6), it(w3, 7),
             it(w3, 8), it(w3, 9), it(w3, 10), it(w3, 11),
             it(w3, 12), it(w3, 13), it(w3, 14), it(w3, 15),
             ("end", w3.finalize)],
        ]
        carry = None
        for kb in range(4):
            if kb == 1:
                wins0[0].stagger = 2   # V exists now; normal trailing ctx
            if kb == 0:
                extra = [lambda: nc.gpsimd.dma_start(bvrow, bv[None, :])]
                extra += [lambda m=m: load_w_slice("wv", wvr, m)
                          for m in range(MT)]
                preps = x_chunk_preps(1, extra)
            elif kb == 1:
                preps = x_chunk_preps(2, [
                    lambda: load_w_slice("wq", wqr, 1),
                    lambda: load_w_slice("wk", wkr, 1),
                ])
            elif kb == 2:
                preps = x_chunk_preps(3, [
                    lambda: load_w_slice("wq", wqr, 2),
                    lambda: load_w_slice("wk", wkr, 2),
                    lambda: load_w_slice("wq", wqr, 3),
                    lambda: load_w_slice("wk", wkr, 3),
                ])
            else:
                # pair 1 needs its FULL kt before its first window's k-loop
                # completes, so all four chunks project here
                preps = []
                for qc2 in range(QC):
                    g = qk_proj_gen(1, qc2)
                    preps += [lambda g=g: next(g, None)] * 5
            carry = run_phase(phase_events[kb], preps, carry_in=carry,
                              carry_delay=6 if kb == 1 else 2)

        # pairs 1..3: plain full-k windows, next pair's projections spread in
        for pr in range(1, MT):
            for qc in range(QC):
                w = Win(pr, qc)
                events = [("item", lambda w=w, kti=kti: w.item(kti))
                          for kti in range(NQT)]
                events.append(("end", w.finalize))
                preps = []
                if pr < MT - 1:
                    g = qk_proj_gen(pr + 1, qc)
                    preps = [lambda g=g: next(g, None)] * 5
                last = (pr == MT - 1 and qc == QC - 1)
                carry = run_phase(events, preps, carry_in=carry,
                                  hold_last=not last, prep_frac=0.6)
        if carry is not None:
            carry()


def build_program():
    from concourse import bacc

    nc = bacc.Bacc("TRN2", target_bir_lowering=False, debug=False)
    hs = nc.dram_tensor("hs", [S, E], F32, kind="ExternalInput").ap()
    wq = nc.dram_tensor("wq", [E, CE], F32, kind="ExternalInput").ap()
    bq = nc.dram_tensor("bq", [CE], F32, kind="ExternalInput").ap()
    wk = nc.dram_tensor("wk", [E, CE], F32, kind="ExternalInput").ap()
    bk = nc.dram_tensor("bk", [CE], F32, kind="ExternalInput").ap()
    wv = nc.dram_tensor("wv", [E, CE], F32, kind="ExternalInput").ap()
    bv = nc.dram_tensor("bv", [CE], F32, kind="ExternalInput").ap()
    out = nc.dram_tensor("out", [S, CE], F32, kind="ExternalOutput").ap()
    with tile.TileContext(nc) as tc:
        _build(tc, out, hs, wq, bq, wk, bk, wv, bv)
    nc.compile()
    return nc


def make_in_maps(inputs):
    """Slice full inputs into 8 per-core input maps."""
    hsf = np.ascontiguousarray(np.asarray(inputs["hidden_states"], dtype=np.float32))
    w = {k: np.asarray(inputs[k], dtype=np.float32) for k in
         ("Wq", "bq", "Wk", "bk", "Wv", "bv")}
    in_maps = []
    for core in range(NCORES):
        b, hg = core // HG, core % HG
        cols = slice(hg * CE, (hg + 1) * CE)
        in_maps.append({
            "hs": hsf[b],
            "wq": np.ascontiguousarray(w["Wq"][:, cols]),
            "bq": np.ascontiguousarray(w["bq"][cols]),
            "wk": np.ascontiguousarray(w["Wk"][:, cols]),
            "bk": np.ascontiguousarray(w["bk"][cols]),
            "wv": np.ascontiguousarray(w["Wv"][:, cols]),
            "bv": np.ascontiguousarray(w["bv"][cols]),
        })
    return in_maps


def assemble(results):
    """Gather 8 per-core [S, CE] outputs into the full [B, S, E] output."""
    full = np.empty((B, S, E), dtype=np.float32)
    for core in range(NCORES):
        b, hg = core // HG, core % HG
        full[b, :, hg * CE : (hg + 1) * CE] = results[core]["out"]
    return full


_NC_CACHE = None


def kernel(**inputs):
    global _NC_CACHE
    from concourse.bass_utils import run_bass_kernel_spmd

    if _NC_CACHE is None:
        _NC_CACHE = build_program()
    res = run_bass_kernel_spmd(_NC_CACHE, make_in_maps(inputs),
                               core_ids=list(range(NCORES)))
    return assemble(res.results)


# revision 40
# speedup vs baseline: 1.0081x; 1.0081x over previous
"""Multi-head attention (B=4, S=2048, E=1024, H=16, D=64) on 8 trn2 cores.

Sharding: core c handles batch b=c//2 and head-group hg=c%2 (8 heads, 512
embed cols). QKV projection weights are column-sharded by head group so
attention is fully local per device.

Per-core plan (bf16 matmul operands, fp32 PSUM accumulation):
  - The ACT (scalar) engine's exp stream is the roofline: 256 activations
    of [128, 1024] at ~1.11us each (~285us busy). The schedule exists to
    start that stream early and keep it fed; measured steady state runs
    at the ACT roofline (PE ~88% busy under it).
  - All casting loads (X s-tiles, W m-slices) ride the gpsimd cast-DMA
    queue, ordered so qk_proj(0,0)'s inputs land first. X is transposed
    into xt[e][:, s] with REGULAR matmuls against a bf16 identity (not
    transpose-mode) so the PE HAM clock-gate warms up from ~9us.
  - pair 0's four q-chunk windows run k-BLOCKED: in phase kb, window qc
    processes the k-tiles unlocked by X-chunk kb while X-chunk kb+1
    loads/transposes/projects in the PE gaps. Suspended windows park
    their ctx PSUM accumulator in SBUF (DVE copy / in-place add) and
    resume next phase, so only one window's ctx pair is PSUM-live.
  - pairs 1..3 run plain full-k windows with the next pair's Q/K
    projections spread between attention items.
  - scores^T[k,q] = KT.T @ QT per head-pair: two K=64 matmuls row-tiled
    at partition bases 0/64 (concurrent in the PE array).
  - ctx^T_aug[65,q] += [V|1].T @ exp ; row 64 = softmax denominator Z.
    ctx matmuls trail the exp stream by 2 items (fully deferred in phase
    0 where V isn't projected yet); segment-end flush/suspend/finalize
    ops are delayed past the first 2 items of the NEXT segment so the PE
    queue never blocks on the last exp of a window.
  - output: DVE 32x32 block-transpose + per-q 1/Z scale (Z columns via a
    tiny DRAM bounce; gather on the gpsimd queue) + block-permuted DMA.
  - PSUM budget (8 banks): scores 2x[128,1024]f32 (4) + ctx 2x[65,512]f32
    (2) + shared proj/transpose pool 2x[128,512]f32 (2).
"""

import numpy as np
from contextlib import ExitStack

import concourse.bass as bass
import concourse.mybir as mybir
import concourse.tile as tile
from concourse.bass import ts, ds
from concourse.masks import make_identity

B, S, E = 4, 2048, 1024
H, DH = 16, 64
NCORES = 8
HG = 2                # head groups per batch (cores per batch element)
HPC = H // HG         # heads per core = 8
CE = HPC * DH         # embed cols per core = 512
P = 128
NQT = S // P          # 16 s-tiles of 128
QC = 4                # q chunks of 512
ET = E // P           # 8 e-tiles
MT = CE // P          # 4 output dim tiles (head pairs)

F32 = mybir.dt.float32
BF16 = mybir.dt.bfloat16
AF = mybir.ActivationFunctionType
ADD = mybir.AluOpType.add


def _build(tc, out, hs, wq, bq, wk, bk, wv, bv):
    nc = tc.nc
    with ExitStack() as ctx:
        persist = ctx.enter_context(tc.tile_pool(name="persist", bufs=1))
        xsp = ctx.enter_context(tc.tile_pool(name="xs_pool", bufs=4))
        ep = ctx.enter_context(tc.tile_pool(name="e_pool", bufs=8))
        cp = ctx.enter_context(tc.tile_pool(name="c_pool", bufs=4))
        otp = ctx.enter_context(tc.tile_pool(name="ot_pool", bufs=3))
        zp = ctx.enter_context(tc.tile_pool(name="z_pool", bufs=2))
        pp = ctx.enter_context(tc.tile_pool(name="part_pool", bufs=1))
        drp = ctx.enter_context(tc.tile_pool(name="dram_pool", bufs=2, space="DRAM"))
        # PSUM: proj/transpose shared pool (2 banks) + scores (4) + ctx (2)
        pjp = ctx.enter_context(tc.tile_pool(name="proj_psum", bufs=2, space="PSUM"))
        spp = ctx.enter_context(tc.tile_pool(name="s_psum", bufs=2, space="PSUM"))
        cpp = ctx.enter_context(tc.tile_pool(name="ctx_psum", bufs=2, space="PSUM"))

        # ---- persistent buffers ----
        qt = [persist.tile([P, S], BF16, tag=f"qt{m}", name=f"qt{m}")
              for m in range(MT)]
        kt = [persist.tile([P, S], BF16, tag=f"kt{m}", name=f"kt{m}")
              for m in range(MT)]
        v = [persist.tile([P, HPC, DH + 1], BF16, tag=f"v{st}", name=f"v{st}")
             for st in range(NQT)]
        xt = [persist.tile([P, S], BF16, tag=f"xt{e}", name=f"xt{e}")
              for e in range(ET)]
        bqs = persist.tile([P, MT], F32, tag="bqs")
        bks = persist.tile([P, MT], F32, tag="bks")
        bvrow = persist.tile([1, CE], BF16, tag="bvrow")
        ones_row = persist.tile([1, P], BF16, tag="ones_row")
        ones_col = persist.tile([P, HPC], BF16, tag="ones_col")
        ident = persist.tile([P, P], BF16, tag="ident")
        ws = {}
        for nm in ("wq", "wk", "wv"):
            ws[nm] = persist.tile([P, ET, CE], BF16, tag=nm, name=nm)

        wqr = wq.rearrange("(o p) c -> p o c", p=P)
        wkr = wk.rearrange("(o p) c -> p o c", p=P)
        wvr = wv.rearrange("(o p) c -> p o c", p=P)
        hsr = hs.rearrange("(t p) e -> p t e", p=P)  # [128, 16, 1024]

        # ---- building blocks ----
        def load_w_slice(nm, wsrc, m):
            nc.gpsimd.dma_start(ws[nm][:, :, ts(m, P)], wsrc[:, :, ts(m, P)])

        def load_x_dma(s):
            """gpsimd cast-DMA (fp32->bf16). The first tiles go as
            half-tiles so the first transposes unblock sooner."""
            xs_t = xsp.tile([P, E], BF16, tag="xs", name="xs")
            if s < 2:
                nc.gpsimd.dma_start(xs_t[:, 0:512], hsr[:, s, 0:512])
                nc.gpsimd.dma_start(xs_t[:, 512:1024], hsr[:, s, 512:1024])
            else:
                nc.gpsimd.dma_start(xs_t, hsr[:, s, :])
            return xs_t

        def warm_mm(n=1):
            """Full-array (K=128) matmuls into a never-read scratch tile.
            The HAM clock gate tracks real PE activity: K=1 dummies don't
            register, and transpose-mode doesn't either. A burst during
            the initial DMA wait unthrottles the PE to 2.4GHz; later
            singles keep the MID window from seeing a fully-idle 3.4us."""
            scratch = spp.tile([P, 1024], F32, tag="sps", name="warm")
            for _ in range(n):
                nc.tensor.matmul(scratch[:, 0:128], lhsT=ident,
                                 rhs=ident, start=True, stop=True)

        def transpose_half(xs_t, s, eg):
            # regular matmul vs identity (counts as PE-busy for HAM, unlike
            # transpose-mode); fp32 PSUM out, cast on the DVE copy
            tp = pjp.tile([P, 4, P], F32, tag="pps", name="tps")
            for j in range(4):
                nc.tensor.matmul(tp[:, j, :],
                                 lhsT=xs_t[:, ts(eg * 4 + j, P)],
                                 rhs=ident, start=True, stop=True)
            for j in range(4):
                nc.vector.tensor_copy(out=xt[eg * 4 + j][:, ts(s, P)],
                                      in_=tp[:, j, :])

        def v_proj_half(st, half, state):
            """V projection for s-tile st, e-tiles [4*half, 4*half+4)."""
            if half == 0:
                state["ps"] = pjp.tile([P, 512], F32, tag="pps", name="pps")
            ps = state["ps"]
            for e in range(4 * half, 4 * half + 4):
                nc.tensor.matmul(ps, lhsT=xt[e][:, ts(st, P)],
                                 rhs=ws["wv"][:, e, :],
                                 start=(e == 0), stop=False)
            if half == 1:
                nc.tensor.matmul(ps, lhsT=ones_row, rhs=bvrow,
                                 start=False, stop=True)
                nc.vector.tensor_copy(
                    out=v[st][:, :, 0:DH],
                    in_=ps.rearrange("p (h d) -> p h d", h=HPC),
                )
                nc.vector.tensor_copy(out=v[st][:, :, DH], in_=ones_col)

        def v_proj(st):
            state = {}
            v_proj_half(st, 0, state)
            v_proj_half(st, 1, state)

        def qk_proj_gen(m, qc):
            """Q/K projections for dim-tile m, q-chunk qc; yields per 4 e-MMs."""
            for wname, dstt, bias in (("wq", qt, bqs), ("wk", kt, bks)):
                ps = pjp.tile([P, 512], F32, tag="pps", name="pps")
                for e in range(ET):
                    nc.tensor.matmul(
                        ps,
                        lhsT=ws[wname][:, e, ts(m, P)],
                        rhs=xt[e][:, ts(qc, 512)],
                        start=(e == 0),
                        stop=(e == ET - 1),
                    )
                    if e % 4 == 3:
                        yield
                nc.vector.tensor_scalar_add(
                    dstt[m][:, ts(qc, 512)], ps, bias[:, ts(m, 1)]
                )
            yield

        def qk_proj(m, qc):
            for _ in qk_proj_gen(m, qc):
                pass

        # ---- attention window (one (pair, q-chunk)) ----
        class Seg:
            """One PSUM-residency segment of a window's ctx accumulation."""
            __slots__ = ("pends", "ctxA", "ctxB", "first")

            def __init__(self):
                self.pends = []
                self.ctxA = self.ctxB = None
                self.first = False

        class Win:
            def __init__(self, pr, qc):
                self.pr, self.qc = pr, qc
                self.hA, self.hB = 2 * pr, 2 * pr + 1
                self.seg = Seg()
                self.stagger = 2    # None = defer all until flush
                self.pA = self.pB = None

            def _ctx_mm(self, seg, pk, pe, stop):
                if seg.ctxA is None:
                    # lazy open: ensures the previous segment's suspend
                    # (possibly emitted 2 items into THIS segment) precedes
                    # the slot reuse in emission order
                    seg.ctxA = cpp.tile([DH + 1, 512], F32, tag="ctx", name="ctx")
                    seg.ctxB = cpp.tile([DH + 1, 512], F32, tag="ctx", name="ctx")
                    seg.first = True
                nc.tensor.matmul(seg.ctxA, lhsT=v[pk][:, self.hA, :],
                                 rhs=pe[:, 0:512],
                                 start=seg.first, stop=stop)
                nc.tensor.matmul(seg.ctxB, lhsT=v[pk][:, self.hB, :],
                                 rhs=pe[:, 512:1024],
                                 start=seg.first, stop=stop)
                seg.first = False

            def item(self, kti):
                sps = spp.tile([P, 1024], F32, tag="sps", name="sps")
                nc.tensor.matmul(
                    sps[:, 0:512],
                    lhsT=kt[self.pr][0:DH, ts(kti, P)],
                    rhs=qt[self.pr][0:DH, ts(self.qc, 512)],
                    start=True, stop=True,
                )
                nc.tensor.matmul(
                    sps[:, 512:1024],
                    lhsT=kt[self.pr][DH:P, ts(kti, P)],
                    rhs=qt[self.pr][DH:P, ts(self.qc, 512)],
                    start=True, stop=True,
                )
                et = ep.tile([P, 1024], BF16, tag="expT", name="expT")
                nc.scalar.activation(et, sps, AF.Exp, scale=0.125)
                self.seg.pends.append((kti, et))
                if self.stagger is not None and len(self.seg.pends) > self.stagger:
                    self._ctx_mm(self.seg, *self.seg.pends.pop(0), stop=False)

            def _flush(self, seg):
                while seg.pends:
                    pk, pe = seg.pends.pop(0)
                    self._ctx_mm(seg, pk, pe, stop=(not seg.pends))

            def suspend(self):
                """Returns a closure that parks the CURRENT segment in SBUF;
                the window immediately starts a fresh segment so later items
                don't disturb the captured one."""
                seg, self.seg = self.seg, Seg()

                def emit():
                    self._flush(seg)
                    parts = []
                    for ctx_t, part in ((seg.ctxA, self.pA), (seg.ctxB, self.pB)):
                        if part is None:
                            part = pp.tile([DH + 1, 512], F32,
                                           tag=f"part_q{self.qc}_{len(parts)}",
                                           name="part")
                            nc.vector.tensor_copy(out=part, in_=ctx_t)
                        else:
                            nc.vector.tensor_tensor(part, ctx_t, part, ADD)
                        parts.append(part)
                    self.pA, self.pB = parts

                return emit

            def finalize(self):
                """Returns a closure emitting the normalize+store pipeline
                for the captured final segment."""
                seg, self.seg = self.seg, Seg()

                def emit():
                    self._finalize_emit(seg)

                return emit

            def _finalize_emit(self, seg):
                self._flush(seg)
                zd = drp.tile([2, 2, 512], F32, tag="zd", name="zd")
                css = []
                for idx, (ctx_t, part) in enumerate(
                        ((seg.ctxA, self.pA), (seg.ctxB, self.pB))):
                    cs = cp.tile([DH + 1, 512], F32, tag="cs", name="cs")
                    if part is None:
                        nc.vector.tensor_copy(out=cs, in_=ctx_t)
                    else:
                        nc.vector.tensor_tensor(cs, ctx_t, part, ADD)
                    # bounce raw Z through DRAM (twice: avoids step-0 DRAM AP)
                    nc.sync.dma_start(zd[0, idx][None, :], cs[DH : DH + 1, :])
                    nc.sync.dma_start(zd[1, idx][None, :], cs[DH : DH + 1, :])
                    css.append(cs)
                self.ctxA = self.ctxB = None
                # fold Z rows into per-q columns matching the 32x32
                # block-transposed layout, then reciprocal over 64 lanes
                c2 = zp.tile([DH, 2, NQT], F32, tag="c2", name="c2")
                for i in range(2):
                    nc.gpsimd.dma_start(
                        c2[ts(i, 32)],
                        zd[i].rearrange("h (j a) -> a h j", a=32),
                    )
                nc.vector.reciprocal(c2, c2)
                for idx, hl in ((0, self.hA), (1, self.hB)):
                    bt = otp.tile([DH, 512], F32, tag="bt", name="bt")
                    nc.vector.transpose(bt, css[idx][0:DH, :])
                    ot = otp.tile([DH, NQT, 32], F32, tag="ot", name="ot")
                    nc.vector.tensor_tensor(
                        ot,
                        bt.rearrange("p (j b) -> p j b", b=32),
                        c2[:, idx, :, None].to_broadcast([DH, NQT, 32]),
                        mybir.AluOpType.mult,
                    )
                    # block-permuted store: ot[32i+a, j, b] -> row qc*512+32j+a,
                    # col hl*64+32i+b
                    for i in range(2):
                        nc.gpsimd.dma_start(
                            out.rearrange(
                                "(qq j a) (h i b) -> qq h i a j b",
                                j=NQT, a=32, i=2, b=32,
                            )[self.qc, hl, i],
                            ot[ts(i, 32)],
                        )

        # ---- interleaved phase driver with cross-segment end delay ----
        def run_phase(events, preps, carry_in=None, hold_last=True,
                      carry_delay=2, prep_frac=1.0):
            """events: list of ("item", fn) | ("end", factory). At the end
            event's stream position the factory runs (capturing the live
            segment); its emit closure is delayed past the next 2 items so
            the next segment's scores hide the last-exp wait. The final
            emit can carry into the next phase. preps are spread between
            emissions."""
            fns = []
            helds = [[carry_in, carry_delay]] if carry_in is not None else []
            for kind, fn in events:
                if kind == "item":
                    fns.append(fn)
                    for h in helds:
                        h[1] -= 1
                    while helds and helds[0][1] <= 0:
                        fns.append(helds.pop(0)[0])
                else:
                    cell = [None, fn]
                    fns.append(lambda cell=cell: cell.__setitem__(0, cell[1]()))
                    helds.append([lambda cell=cell: cell[0](), 2])
            carry_out = None
            if helds:
                if hold_last:
                    *rest, last = helds
                    fns.extend(h[0] for h in rest)
                    carry_out = last[0]
                else:
                    fns.extend(h[0] for h in helds)
            n_i, n_p = len(fns), len(preps)
            pi = 0
            for i, fn in enumerate(fns):
                fn()
                # prep_frac < 1 front-loads the preps so their trailing DVE
                # (the qk bias) clears before the window-boundary DVE burst
                while pi < n_p and pi * n_i * prep_frac < (i + 1) * n_p:
                    preps[pi]()
                    pi += 1
            while pi < n_p:
                preps[pi]()
                pi += 1
            return carry_out

        def x_chunk_preps(kb, extra_dmas=()):
            """Prep closures for X chunk kb (s-tiles 4kb..4kb+3) + qk(0,kb).
            X DMAs+transposes with the extra (W) DMAs spread between the X
            tiles on the gpsimd queue. V projections are NOT here — they
            ride the next phase's event stream in readiness order."""
            preps = []
            extra_dmas = list(extra_dmas)
            n_ed = len(extra_dmas)
            for si, s in enumerate(range(4 * kb, 4 * kb + 4)):
                box = {}

                def dma(s=s, box=box):
                    box["xs"] = load_x_dma(s)

                preps.append(dma)
                for eg in range(2):
                    preps.append(lambda s=s, eg=eg, box=box:
                                 transpose_half(box["xs"], s, eg))
                # spread the extra W DMAs between the X tiles on the queue
                preps += extra_dmas[si * n_ed // 4:(si + 1) * n_ed // 4]
            g = qk_proj_gen(0, kb)
            preps += [lambda g=g: next(g, None)] * 5
            return preps

        # ---- emission ----
        # prologue. The identity/memsets go first (they occupy the gpsimd
        # engine queue which also dispatches the casting DMAs). Then the
        # two DMA queues race in parallel: gpsimd carries X s0 + all W
        # slices (cast-DMA is gpsimd-only), sync carries fp32 X s1..s3.
        nc.vector.memset(ones_row, 1.0)
        nc.vector.memset(ones_col, 1.0)
        make_identity(nc, ident)
        xs03 = [load_x_dma(s) for s in range(4)]
        load_w_slice("wq", wqr, 0)
        load_w_slice("wk", wkr, 0)
        nc.sync.dma_start(bqs, bq.rearrange("(o p) -> p o", p=P))
        nc.sync.dma_start(bks, bk.rearrange("(o p) -> p o", p=P))
        # warm the HAM clock gate during the DMA wait (PE is idle anyway);
        # 40 MMs ~= 4.3us guarantees one fully-busy free-running window
        warm_mm(40)
        for s in range(4):
            for eg in range(2):
                transpose_half(xs03[s], s, eg)
        qk_proj(0, 0)

        # pair-0 phases: phase kb runs the k-tiles unlocked by chunk kb for
        # windows 0..kb while chunk kb+1 (or pair-1 projections) preps.
        # V projections for chunk c are EVENTS of phase c+1 (where Wv/X have
        # certainly landed), positioned just before their ctx consumers so
        # the in-order PE queue matches runtime readiness.
        wins0 = [Win(0, qc) for qc in range(QC)]
        wins0[0].stagger = None    # phase-0 ctx fully deferred (V not ready)

        def it(w, kti):
            return ("item", lambda w=w, kti=kti: w.item(kti))

        def vp(st):
            return ("item", lambda st=st: v_proj(st))

        w0, w1, w2, w3 = wins0
        phase_events = [
            # phase 0: only chunk-0 scores; ctx deferred into the carried
            # suspend which lands in phase 1 after v0..3
            [it(w0, 0), it(w0, 1), it(w0, 2), it(w0, 3),
             ("end", w0.suspend)],
            # phase 1 (carry_delay=6: held ph0-suspend emits after v3)
            [it(w0, 4), it(w0, 5), vp(0), vp(1), vp(2), vp(3),
             vp(4), it(w0, 6), vp(5), it(w0, 7), vp(6), vp(7),
             ("end", w0.suspend),
             it(w1, 0), it(w1, 1), it(w1, 2), it(w1, 3),
             it(w1, 4), it(w1, 5), it(w1, 6), it(w1, 7),
             ("end", w1.suspend)],
            # phase 2
            [it(w0, 8), it(w0, 9), vp(8), it(w0, 10), vp(9), it(w0, 11),
             vp(10), vp(11), ("end", w0.suspend),
             it(w1, 8), it(w1, 9), it(w1, 10), it(w1, 11),
             ("end", w1.suspend),
             it(w2, 0), it(w2, 1), it(w2, 2), it(w2, 3),
             it(w2, 4), it(w2, 5), it(w2, 6), it(w2, 7),
             it(w2, 8), it(w2, 9), it(w2, 10), it(w2, 11),
             ("end", w2.suspend)],
            # phase 3
            [it(w0, 12), it(w0, 13), vp(12), it(w0, 14), vp(13),
             it(w0, 15), vp(14), vp(15), ("end", w0.finalize),
             it(w1, 12), it(w1, 13), it(w1, 14), it(w1, 15),
             ("end", w1.finalize),
             it(w2, 12), it(w2, 13), it(w2, 14), it(w2, 15),
             ("end", w2.finalize),
             it(w3, 0), it(w3, 1), it(w3, 2), it(w3, 3),
             it(w3, 4), it(w3, 5), it(w3, 6), it(w3, 7),
             it(w3, 8), it(w3, 9), it(w3, 10), it(w3, 11),
             it(w3, 12), it(w3, 13), it(w3, 14), it(w3, 15),
             ("end", w3.finalize)],
        ]
        carry = None
        for kb in range(4):
            if kb == 1:
                wins0[0].stagger = 2   # V exists now; normal trailing ctx
            if kb == 0:
                extra = [lambda: nc.gpsimd.dma_start(bvrow, bv[None, :])]
                extra += [lambda m=m: load_w_slice("wv", wvr, m)
                          for m in range(MT)]
                preps = x_chunk_preps(1, extra)
            elif kb == 1:
                preps = x_chunk_preps(2, [
                    lambda: load_w_slice("wq", wqr, 1),
                    lambda: load_w_slice("wk", wkr, 1),
                ])
            elif kb == 2:
                preps = x_chunk_preps(3, [
                    lambda: load_w_slice("wq", wqr, 2),
                    lambda: load_w_slice("wk", wkr, 2),
                    lambda: load_w_slice("wq", wqr, 3),
                    lambda: load_w_slice("wk", wkr, 3),
                ])
            else:
                # pair 1 needs its FULL kt before its first window's k-loop
                # completes, so all four chunks project here
                preps = []
                for qc2 in range(QC):
                    g = qk_proj_gen(1, qc2)
                    preps += [lambda g=g: next(g, None)] * 5
            carry = run_phase(phase_events[kb], preps, carry_in=carry,
                              carry_delay=6 if kb == 1 else 2)

        # pairs 1..3: plain full-k windows, next pair's projections spread in
        for pr in range(1, MT):
            for qc in range(QC):
                w = Win(pr, qc)
                events = [("item", lambda w=w, kti=kti: w.item(kti))
                          for kti in range(NQT)]
                events.append(("end", w.finalize))
                preps = []
                if pr < MT - 1:
                    g = qk_proj_gen(pr + 1, qc)
                    preps = [lambda g=g: next(g, None)] * 5
                last = (pr == MT - 1 and qc == QC - 1)
                carry = run_phase(events, preps, carry_in=carry,
                                  hold_last=not last)
        if carry is not None:
            carry()


def build_program():
    from concourse import bacc

    nc = bacc.Bacc("TRN2", target_bir_lowering=False, debug=False)
    hs = nc.dram_tensor("hs", [S, E], F32, kind="ExternalInput").ap()
    wq = nc.dram_tensor("wq", [E, CE], F32, kind="ExternalInput").ap()
    bq = nc.dram_tensor("bq", [CE], F32, kind="ExternalInput").ap()
    wk = nc.dram_tensor("wk", [E, CE], F32, kind="ExternalInput").ap()
    bk = nc.dram_tensor("bk", [CE], F32, kind="ExternalInput").ap()
    wv = nc.dram_tensor("wv", [E, CE], F32, kind="ExternalInput").ap()
    bv = nc.dram_tensor("bv", [CE], F32, kind="ExternalInput").ap()
    out = nc.dram_tensor("out", [S, CE], F32, kind="ExternalOutput").ap()
    with tile.TileContext(nc) as tc:
        _build(tc, out, hs, wq, bq, wk, bk, wv, bv)
    nc.compile()
    return nc


def make_in_maps(inputs):
    """Slice full inputs into 8 per-core input maps."""
    hsf = np.ascontiguousarray(np.asarray(inputs["hidden_states"], dtype=np.float32))
    w = {k: np.asarray(inputs[k], dtype=np.float32) for k in
         ("Wq", "bq", "Wk", "bk", "Wv", "bv")}
    in_maps = []
    for core in range(NCORES):
        b, hg = core // HG, core % HG
        cols = slice(hg * CE, (hg + 1) * CE)
        in_maps.append({
            "hs": hsf[b],
            "wq": np.ascontiguousarray(w["Wq"][:, cols]),
            "bq": np.ascontiguousarray(w["bq"][cols]),
            "wk": np.ascontiguousarray(w["Wk"][:, cols]),
            "bk": np.ascontiguousarray(w["bk"][cols]),
            "wv": np.ascontiguousarray(w["Wv"][:, cols]),
            "bv": np.ascontiguousarray(w["bv"][cols]),
        })
    return in_maps


def assemble(results):
    """Gather 8 per-core [S, CE] outputs into the full [B, S, E] output."""
    full = np.empty((B, S, E), dtype=np.float32)
    for core in range(NCORES):
        b, hg = core // HG, core % HG
        full[b, :, hg * CE : (hg + 1) * CE] = results[core]["out"]
    return full


_NC_CACHE = None


def kernel(**inputs):
    global _NC_CACHE
    from concourse.bass_utils import run_bass_kernel_spmd

    if _NC_CACHE is None:
        _NC_CACHE = build_program()
    res = run_bass_kernel_spmd(_NC_CACHE, make_in_maps(inputs),
                               core_ids=list(range(NCORES)))
    return assemble(res.results)


# revision 43
# speedup vs baseline: 1.0083x; 1.0003x over previous
"""Multi-head attention (B=4, S=2048, E=1024, H=16, D=64) on 8 trn2 cores.

Sharding: core c handles batch b=c//2 and head-group hg=c%2 (8 heads, 512
embed cols). QKV projection weights are column-sharded by head group so
attention is fully local per device.

Per-core plan (bf16 matmul operands, fp32 PSUM accumulation):
  - The ACT (scalar) engine's exp stream is the roofline: 256 activations
    of [128, 1024] at ~1.11us each (~285us busy). The schedule exists to
    start that stream early and keep it fed; measured steady state runs
    at the ACT roofline (PE ~88% busy under it).
  - All casting loads (X s-tiles, W m-slices) ride the gpsimd cast-DMA
    queue, ordered so qk_proj(0,0)'s inputs land first. X is transposed
    into xt[e][:, s] with REGULAR matmuls against a bf16 identity (not
    transpose-mode) so the PE HAM clock-gate warms up from ~9us.
  - pair 0's four q-chunk windows run k-BLOCKED: in phase kb, window qc
    processes the k-tiles unlocked by X-chunk kb while X-chunk kb+1
    loads/transposes/projects in the PE gaps. Suspended windows park
    their ctx PSUM accumulator in SBUF (DVE copy / in-place add) and
    resume next phase, so only one window's ctx pair is PSUM-live.
  - pairs 1..3 run plain full-k windows with the next pair's Q/K
    projections spread between attention items.
  - scores^T[k,q] = KT.T @ QT per head-pair: two K=64 matmuls row-tiled
    at partition bases 0/64 (concurrent in the PE array).
  - ctx^T_aug[65,q] += [V|1].T @ exp ; row 64 = softmax denominator Z.
    ctx matmuls trail the exp stream by 2 items (fully deferred in phase
    0 where V isn't projected yet); segment-end flush/suspend/finalize
    ops are delayed past the first 2 items of the NEXT segment so the PE
    queue never blocks on the last exp of a window.
  - output: DVE 32x32 block-transpose + per-q 1/Z scale (Z columns via a
    tiny DRAM bounce; gather on the gpsimd queue) + block-permuted DMA.
  - PSUM budget (8 banks): scores 2x[128,1024]f32 (4) + ctx 2x[65,512]f32
    (2) + shared proj/transpose pool 2x[128,512]f32 (2).
"""

import numpy as np
from contextlib import ExitStack

import concourse.bass as bass
import concourse.mybir as mybir
import concourse.tile as tile
from concourse.bass import ts, ds
from concourse.masks import make_identity

B, S, E = 4, 2048, 1024
H, DH = 16, 64
NCORES = 8
HG = 2                # head groups per batch (cores per batch element)
HPC = H // HG         # heads per core = 8
CE = HPC * DH         # embed cols per core = 512
P = 128
NQT = S // P          # 16 s-tiles of 128
QC = 4                # q chunks of 512
ET = E // P           # 8 e-tiles
MT = CE // P          # 4 output dim tiles (head pairs)

F32 = mybir.dt.float32
BF16 = mybir.dt.bfloat16
AF = mybir.ActivationFunctionType
ADD = mybir.AluOpType.add


def _build(tc, out, hs, wq, bq, wk, bk, wv, bv):
    nc = tc.nc
    with ExitStack() as ctx:
        persist = ctx.enter_context(tc.tile_pool(name="persist", bufs=1))
        xsp = ctx.enter_context(tc.tile_pool(name="xs_pool", bufs=4))
        ep = ctx.enter_context(tc.tile_pool(name="e_pool", bufs=8))
        cp = ctx.enter_context(tc.tile_pool(name="c_pool", bufs=4))
        otp = ctx.enter_context(tc.tile_pool(name="ot_pool", bufs=3))
        zp = ctx.enter_context(tc.tile_pool(name="z_pool", bufs=2))
        pp = ctx.enter_context(tc.tile_pool(name="part_pool", bufs=1))
        drp = ctx.enter_context(tc.tile_pool(name="dram_pool", bufs=2, space="DRAM"))
        # PSUM: proj/transpose shared pool (2 banks) + scores (4) + ctx (2)
        pjp = ctx.enter_context(tc.tile_pool(name="proj_psum", bufs=2, space="PSUM"))
        spp = ctx.enter_context(tc.tile_pool(name="s_psum", bufs=2, space="PSUM"))
        cpp = ctx.enter_context(tc.tile_pool(name="ctx_psum", bufs=2, space="PSUM"))

        # ---- persistent buffers ----
        qt = [persist.tile([P, S], BF16, tag=f"qt{m}", name=f"qt{m}")
              for m in range(MT)]
        kt = [persist.tile([P, S], BF16, tag=f"kt{m}", name=f"kt{m}")
              for m in range(MT)]
        v = [persist.tile([P, HPC, DH + 1], BF16, tag=f"v{st}", name=f"v{st}")
             for st in range(NQT)]
        xt = [persist.tile([P, S], BF16, tag=f"xt{e}", name=f"xt{e}")
              for e in range(ET)]
        bqs = persist.tile([P, MT], F32, tag="bqs")
        bks = persist.tile([P, MT], F32, tag="bks")
        bvrow = persist.tile([1, CE], BF16, tag="bvrow")
        ones_row = persist.tile([1, P], BF16, tag="ones_row")
        ones_col = persist.tile([P, HPC], BF16, tag="ones_col")
        ident = persist.tile([P, P], BF16, tag="ident")
        ws = {}
        for nm in ("wq", "wk", "wv"):
            ws[nm] = persist.tile([P, ET, CE], BF16, tag=nm, name=nm)

        wqr = wq.rearrange("(o p) c -> p o c", p=P)
        wkr = wk.rearrange("(o p) c -> p o c", p=P)
        wvr = wv.rearrange("(o p) c -> p o c", p=P)
        hsr = hs.rearrange("(t p) e -> p t e", p=P)  # [128, 16, 1024]

        # ---- building blocks ----
        def load_w_slice(nm, wsrc, m):
            nc.gpsimd.dma_start(ws[nm][:, :, ts(m, P)], wsrc[:, :, ts(m, P)])

        def load_x_dma(s):
            """gpsimd cast-DMA (fp32->bf16). The first tiles go as
            half-tiles so the first transposes unblock sooner."""
            xs_t = xsp.tile([P, E], BF16, tag="xs", name="xs")
            if s < 2:
                nc.gpsimd.dma_start(xs_t[:, 0:512], hsr[:, s, 0:512])
                nc.gpsimd.dma_start(xs_t[:, 512:1024], hsr[:, s, 512:1024])
            else:
                nc.gpsimd.dma_start(xs_t, hsr[:, s, :])
            return xs_t

        def warm_mm(n=1):
            """Full-array (K=128) matmuls into a never-read scratch tile.
            The HAM clock gate tracks real PE activity: K=1 dummies don't
            register, and transpose-mode doesn't either. A burst during
            the initial DMA wait unthrottles the PE to 2.4GHz; later
            singles keep the MID window from seeing a fully-idle 3.4us."""
            scratch = spp.tile([P, 1024], F32, tag="sps", name="warm")
            for _ in range(n):
                nc.tensor.matmul(scratch[:, 0:128], lhsT=ident,
                                 rhs=ident, start=True, stop=True)

        def transpose_half(xs_t, s, eg):
            # regular matmul vs identity (counts as PE-busy for HAM, unlike
            # transpose-mode); fp32 PSUM out, cast on the DVE copy
            tp = pjp.tile([P, 4, P], F32, tag="pps", name="tps")
            for j in range(4):
                nc.tensor.matmul(tp[:, j, :],
                                 lhsT=xs_t[:, ts(eg * 4 + j, P)],
                                 rhs=ident, start=True, stop=True)
            for j in range(4):
                nc.vector.tensor_copy(out=xt[eg * 4 + j][:, ts(s, P)],
                                      in_=tp[:, j, :])

        def v_proj_half(st, half, state):
            """V projection for s-tile st, e-tiles [4*half, 4*half+4)."""
            if half == 0:
                state["ps"] = pjp.tile([P, 512], F32, tag="pps", name="pps")
            ps = state["ps"]
            for e in range(4 * half, 4 * half + 4):
                nc.tensor.matmul(ps, lhsT=xt[e][:, ts(st, P)],
                                 rhs=ws["wv"][:, e, :],
                                 start=(e == 0), stop=False)
            if half == 1:
                nc.tensor.matmul(ps, lhsT=ones_row, rhs=bvrow,
                                 start=False, stop=True)
                nc.vector.tensor_copy(
                    out=v[st][:, :, 0:DH],
                    in_=ps.rearrange("p (h d) -> p h d", h=HPC),
                )
                nc.vector.tensor_copy(out=v[st][:, :, DH], in_=ones_col)

        def v_proj(st):
            state = {}
            v_proj_half(st, 0, state)
            v_proj_half(st, 1, state)

        def qk_proj_gen(m, qc):
            """Q/K projections for dim-tile m, q-chunk qc; yields per 4 e-MMs."""
            for wname, dstt, bias in (("wq", qt, bqs), ("wk", kt, bks)):
                ps = pjp.tile([P, 512], F32, tag="pps", name="pps")
                for e in range(ET):
                    nc.tensor.matmul(
                        ps,
                        lhsT=ws[wname][:, e, ts(m, P)],
                        rhs=xt[e][:, ts(qc, 512)],
                        start=(e == 0),
                        stop=(e == ET - 1),
                    )
                    if e % 4 == 3:
                        yield
                nc.vector.tensor_scalar_add(
                    dstt[m][:, ts(qc, 512)], ps, bias[:, ts(m, 1)]
                )
            yield

        def qk_proj(m, qc):
            for _ in qk_proj_gen(m, qc):
                pass

        # ---- attention window (one (pair, q-chunk)) ----
        class Seg:
            """One PSUM-residency segment of a window's ctx accumulation."""
            __slots__ = ("pends", "ctxA", "ctxB", "first")

            def __init__(self):
                self.pends = []
                self.ctxA = self.ctxB = None
                self.first = False

        class Win:
            def __init__(self, pr, qc):
                self.pr, self.qc = pr, qc
                self.hA, self.hB = 2 * pr, 2 * pr + 1
                self.seg = Seg()
                self.stagger = 2    # None = defer all until flush
                self.pA = self.pB = None

            def _ctx_mm(self, seg, pk, pe, stop):
                if seg.ctxA is None:
                    # lazy open: ensures the previous segment's suspend
                    # (possibly emitted 2 items into THIS segment) precedes
                    # the slot reuse in emission order
                    seg.ctxA = cpp.tile([DH + 1, 512], F32, tag="ctx", name="ctx")
                    seg.ctxB = cpp.tile([DH + 1, 512], F32, tag="ctx", name="ctx")
                    seg.first = True
                nc.tensor.matmul(seg.ctxA, lhsT=v[pk][:, self.hA, :],
                                 rhs=pe[:, 0:512],
                                 start=seg.first, stop=stop)
                nc.tensor.matmul(seg.ctxB, lhsT=v[pk][:, self.hB, :],
                                 rhs=pe[:, 512:1024],
                                 start=seg.first, stop=stop)
                seg.first = False

            def item(self, kti):
                sps = spp.tile([P, 1024], F32, tag="sps", name="sps")
                nc.tensor.matmul(
                    sps[:, 0:512],
                    lhsT=kt[self.pr][0:DH, ts(kti, P)],
                    rhs=qt[self.pr][0:DH, ts(self.qc, 512)],
                    start=True, stop=True,
                )
                nc.tensor.matmul(
                    sps[:, 512:1024],
                    lhsT=kt[self.pr][DH:P, ts(kti, P)],
                    rhs=qt[self.pr][DH:P, ts(self.qc, 512)],
                    start=True, stop=True,
                )
                et = ep.tile([P, 1024], BF16, tag="expT", name="expT")
                nc.scalar.activation(et, sps, AF.Exp, scale=0.125)
                self.seg.pends.append((kti, et))
                if self.stagger is not None and len(self.seg.pends) > self.stagger:
                    self._ctx_mm(self.seg, *self.seg.pends.pop(0), stop=False)

            def _flush(self, seg):
                while seg.pends:
                    pk, pe = seg.pends.pop(0)
                    self._ctx_mm(seg, pk, pe, stop=(not seg.pends))

            def suspend(self):
                """Returns a closure that parks the CURRENT segment in SBUF;
                the window immediately starts a fresh segment so later items
                don't disturb the captured one."""
                seg, self.seg = self.seg, Seg()

                def emit():
                    self._flush(seg)
                    parts = []
                    for ctx_t, part in ((seg.ctxA, self.pA), (seg.ctxB, self.pB)):
                        if part is None:
                            part = pp.tile([DH + 1, 512], F32,
                                           tag=f"part_q{self.qc}_{len(parts)}",
                                           name="part")
                            nc.vector.tensor_copy(out=part, in_=ctx_t)
                        else:
                            nc.vector.tensor_tensor(part, ctx_t, part, ADD)
                        parts.append(part)
                    self.pA, self.pB = parts

                return emit

            def finalize(self):
                """Returns a closure emitting the normalize+store pipeline
                for the captured final segment."""
                seg, self.seg = self.seg, Seg()

                def emit():
                    self._finalize_emit(seg)

                return emit

            def _finalize_emit(self, seg):
                self._flush(seg)
                zd = drp.tile([2, 512], F32, tag="zd", name="zd")
                css = []
                for idx, (ctx_t, part) in enumerate(
                        ((seg.ctxA, self.pA), (seg.ctxB, self.pB))):
                    cs = cp.tile([DH + 1, 512], F32, tag="cs", name="cs")
                    if part is None:
                        nc.vector.tensor_copy(out=cs, in_=ctx_t)
                    else:
                        nc.vector.tensor_tensor(cs, ctx_t, part, ADD)
                    # bounce raw Z through DRAM; both c2 gathers below read
                    # this single copy (two separate DMAs, so no step-0 AP)
                    nc.sync.dma_start(zd[idx][None, :], cs[DH : DH + 1, :])
                    css.append(cs)
                self.ctxA = self.ctxB = None
                # fold Z rows into per-q columns matching the 32x32
                # block-transposed layout, then reciprocal over 64 lanes
                c2 = zp.tile([DH, 2, NQT], F32, tag="c2", name="c2")
                for i in range(2):
                    nc.gpsimd.dma_start(
                        c2[ts(i, 32)],
                        zd.rearrange("h (j a) -> a h j", a=32),
                    )
                nc.vector.reciprocal(c2, c2)
                for idx, hl in ((0, self.hA), (1, self.hB)):
                    bt = otp.tile([DH, 512], F32, tag="bt", name="bt")
                    nc.vector.transpose(bt, css[idx][0:DH, :])
                    ot = otp.tile([DH, NQT, 32], F32, tag="ot", name="ot")
                    nc.vector.tensor_tensor(
                        ot,
                        bt.rearrange("p (j b) -> p j b", b=32),
                        c2[:, idx, :, None].to_broadcast([DH, NQT, 32]),
                        mybir.AluOpType.mult,
                    )
                    # block-permuted store: ot[32i+a, j, b] -> row qc*512+32j+a,
                    # col hl*64+32i+b
                    for i in range(2):
                        nc.gpsimd.dma_start(
                            out.rearrange(
                                "(qq j a) (h i b) -> qq h i a j b",
                                j=NQT, a=32, i=2, b=32,
                            )[self.qc, hl, i],
                            ot[ts(i, 32)],
                        )

        # ---- interleaved phase driver with cross-segment end delay ----
        def run_phase(events, preps, carry_in=None, hold_last=True,
                      carry_delay=2, prep_frac=1.0):
            """events: list of ("item", fn) | ("end", factory). At the end
            event's stream position the factory runs (capturing the live
            segment); its emit closure is delayed past the next 2 items so
            the next segment's scores hide the last-exp wait. The final
            emit can carry into the next phase. preps are spread between
            emissions."""
            fns = []
            helds = [[carry_in, carry_delay]] if carry_in is not None else []
            for kind, fn in events:
                if kind == "item":
                    fns.append(fn)
                    for h in helds:
                        h[1] -= 1
                    while helds and helds[0][1] <= 0:
                        fns.append(helds.pop(0)[0])
                else:
                    cell = [None, fn]
                    fns.append(lambda cell=cell: cell.__setitem__(0, cell[1]()))
                    helds.append([lambda cell=cell: cell[0](), 2])
            carry_out = None
            if helds:
                if hold_last:
                    *rest, last = helds
                    fns.extend(h[0] for h in rest)
                    carry_out = last[0]
                else:
                    fns.extend(h[0] for h in helds)
            n_i, n_p = len(fns), len(preps)
            pi = 0
            for i, fn in enumerate(fns):
                fn()
                # prep_frac < 1 front-loads the preps so their trailing DVE
                # (the qk bias) clears before the window-boundary DVE burst
                while pi < n_p and pi * n_i * prep_frac < (i + 1) * n_p:
                    preps[pi]()
                    pi += 1
            while pi < n_p:
                preps[pi]()
                pi += 1
            return carry_out

        def x_chunk_preps(kb, extra_dmas=()):
            """Prep closures for X chunk kb (s-tiles 4kb..4kb+3) + qk(0,kb).
            X DMAs+transposes with the extra (W) DMAs spread between the X
            tiles on the gpsimd queue. V projections are NOT here — they
            ride the next phase's event stream in readiness order."""
            preps = []
            extra_dmas = list(extra_dmas)
            n_ed = len(extra_dmas)
            for si, s in enumerate(range(4 * kb, 4 * kb + 4)):
                box = {}

                def dma(s=s, box=box):
                    box["xs"] = load_x_dma(s)

                preps.append(dma)
                for eg in range(2):
                    preps.append(lambda s=s, eg=eg, box=box:
                                 transpose_half(box["xs"], s, eg))
                # spread the extra W DMAs between the X tiles on the queue
                preps += extra_dmas[si * n_ed // 4:(si + 1) * n_ed // 4]
            g = qk_proj_gen(0, kb)
            preps += [lambda g=g: next(g, None)] * 5
            return preps

        # ---- emission ----
        # prologue. The identity/memsets go first (they occupy the gpsimd
        # engine queue which also dispatches the casting DMAs). Then the
        # two DMA queues race in parallel: gpsimd carries X s0 + all W
        # slices (cast-DMA is gpsimd-only), sync carries fp32 X s1..s3.
        nc.vector.memset(ones_row, 1.0)
        nc.vector.memset(ones_col, 1.0)
        make_identity(nc, ident)
        xs03 = [load_x_dma(s) for s in range(4)]
        load_w_slice("wq", wqr, 0)
        load_w_slice("wk", wkr, 0)
        nc.sync.dma_start(bqs, bq.rearrange("(o p) -> p o", p=P))
        nc.sync.dma_start(bks, bk.rearrange("(o p) -> p o", p=P))
        # warm the HAM clock gate during the DMA wait (PE is idle anyway);
        # 40 MMs ~= 4.3us guarantees one fully-busy free-running window
        warm_mm(40)
        for s in range(4):
            for eg in range(2):
                transpose_half(xs03[s], s, eg)
        qk_proj(0, 0)

        # pair-0 phases: phase kb runs the k-tiles unlocked by chunk kb for
        # windows 0..kb while chunk kb+1 (or pair-1 projections) preps.
        # V projections for chunk c are EVENTS of phase c+1 (where Wv/X have
        # certainly landed), positioned just before their ctx consumers so
        # the in-order PE queue matches runtime readiness.
        wins0 = [Win(0, qc) for qc in range(QC)]
        wins0[0].stagger = None    # phase-0 ctx fully deferred (V not ready)

        def it(w, kti):
            return ("item", lambda w=w, kti=kti: w.item(kti))

        def vp(st):
            return ("item", lambda st=st: v_proj(st))

        w0, w1, w2, w3 = wins0
        phase_events = [
            # phase 0: only chunk-0 scores; ctx deferred into the carried
            # suspend which lands in phase 1 after v0..3
            [it(w0, 0), it(w0, 1), it(w0, 2), it(w0, 3),
             ("end", w0.suspend)],
            # phase 1 (carry_delay=6: held ph0-suspend emits after v3)
            [it(w0, 4), it(w0, 5), vp(0), vp(1), vp(2), vp(3),
             vp(4), it(w0, 6), vp(5), it(w0, 7), vp(6), vp(7),
             ("end", w0.suspend),
             it(w1, 0), it(w1, 1), it(w1, 2), it(w1, 3),
             it(w1, 4), it(w1, 5), it(w1, 6), it(w1, 7),
             ("end", w1.suspend)],
            # phase 2
            [it(w0, 8), it(w0, 9), vp(8), it(w0, 10), vp(9), it(w0, 11),
             vp(10), vp(11), ("end", w0.suspend),
             it(w1, 8), it(w1, 9), it(w1, 10), it(w1, 11),
             ("end", w1.suspend),
             it(w2, 0), it(w2, 1), it(w2, 2), it(w2, 3),
             it(w2, 4), it(w2, 5), it(w2, 6), it(w2, 7),
             it(w2, 8), it(w2, 9), it(w2, 10), it(w2, 11),
             ("end", w2.suspend)],
            # phase 3
            [it(w0, 12), it(w0, 13), vp(12), it(w0, 14), vp(13),
             it(w0, 15), vp(14), vp(15), ("end", w0.finalize),
             it(w1, 12), it(w1, 13), it(w1, 14), it(w1, 15),
             ("end", w1.finalize),
             it(w2, 12), it(w2, 13), it(w2, 14), it(w2, 15),
             ("end", w2.finalize),
             it(w3, 0), it(w3, 1), it(w3, 2), it(w3, 3),
             it(w3, 4), it(w3, 5), it(w3, 6), it(w3, 7),
             it(w3, 8), it(w3, 9), it(w3, 10), it(w3, 11),
             it(w3, 12), it(w3, 13), it(w3, 14), it(w3, 15),
             ("end", w3.finalize)],
        ]
        carry = None
        for kb in range(4):
            if kb == 1:
                wins0[0].stagger = 2   # V exists now; normal trailing ctx
            if kb == 0:
                extra = [lambda: nc.gpsimd.dma_start(bvrow, bv[None, :])]
                extra += [lambda m=m: load_w_slice("wv", wvr, m)
                          for m in range(MT)]
                preps = x_chunk_preps(1, extra)
            elif kb == 1:
                preps = x_chunk_preps(2, [
                    lambda: load_w_slice("wq", wqr, 1),
                    lambda: load_w_slice("wk", wkr, 1),
                ])
            elif kb == 2:
                preps = x_chunk_preps(3, [
                    lambda: load_w_slice("wq", wqr, 2),
                    lambda: load_w_slice("wk", wkr, 2),
                    lambda: load_w_slice("wq", wqr, 3),
                    lambda: load_w_slice("wk", wkr, 3),
                ])
            else:
                # pair 1 needs its FULL kt before its first window's k-loop
                # completes, so all four chunks project here
                preps = []
                for qc2 in range(QC):
                    g = qk_proj_gen(1, qc2)
                    preps += [lambda g=g: next(g, None)] * 5
            carry = run_phase(phase_events[kb], preps, carry_in=carry,
                              carry_delay=6 if kb == 1 else 2)

        # pairs 1..3: plain full-k windows, next pair's projections spread in
        for pr in range(1, MT):
            for qc in range(QC):
                w = Win(pr, qc)
                events = [("item", lambda w=w, kti=kti: w.item(kti))
                          for kti in range(NQT)]
                events.append(("end", w.finalize))
                preps = []
                if pr < MT - 1:
                    g = qk_proj_gen(pr + 1, qc)
                    preps = [lambda g=g: next(g, None)] * 5
                last = (pr == MT - 1 and qc == QC - 1)
                carry = run_phase(events, preps, carry_in=carry,
                                  hold_last=not last)
        if carry is not None:
            carry()


def build_program():
    from concourse import bacc

    nc = bacc.Bacc("TRN2", target_bir_lowering=False, debug=False)
    hs = nc.dram_tensor("hs", [S, E], F32, kind="ExternalInput").ap()
    wq = nc.dram_tensor("wq", [E, CE], F32, kind="ExternalInput").ap()
    bq = nc.dram_tensor("bq", [CE], F32, kind="ExternalInput").ap()
    wk = nc.dram_tensor("wk", [E, CE], F32, kind="ExternalInput").ap()
    bk = nc.dram_tensor("bk", [CE], F32, kind="ExternalInput").ap()
    wv = nc.dram_tensor("wv", [E, CE], F32, kind="ExternalInput").ap()
    bv = nc.dram_tensor("bv", [CE], F32, kind="ExternalInput").ap()
    out = nc.dram_tensor("out", [S, CE], F32, kind="ExternalOutput").ap()
    with tile.TileContext(nc) as tc:
        _build(tc, out, hs, wq, bq, wk, bk, wv, bv)
    nc.compile()
    return nc


def make_in_maps(inputs):
    """Slice full inputs into 8 per-core input maps."""
    hsf = np.ascontiguousarray(np.asarray(inputs["hidden_states"], dtype=np.float32))
    w = {k: np.asarray(inputs[k], dtype=np.float32) for k in
         ("Wq", "bq", "Wk", "bk", "Wv", "bv")}
    in_maps = []
    for core in range(NCORES):
        b, hg = core // HG, core % HG
        cols = slice(hg * CE, (hg + 1) * CE)
        in_maps.append({
            "hs": hsf[b],
            "wq": np.ascontiguousarray(w["Wq"][:, cols]),
            "bq": np.ascontiguousarray(w["bq"][cols]),
            "wk": np.ascontiguousarray(w["Wk"][:, cols]),
            "bk": np.ascontiguousarray(w["bk"][cols]),
            "wv": np.ascontiguousarray(w["Wv"][:, cols]),
            "bv": np.ascontiguousarray(w["bv"][cols]),
        })
    return in_maps


def assemble(results):
    """Gather 8 per-core [S, CE] outputs into the full [B, S, E] output."""
    full = np.empty((B, S, E), dtype=np.float32)
    for core in range(NCORES):
        b, hg = core // HG, core % HG
        full[b, :, hg * CE : (hg + 1) * CE] = results[core]["out"]
    return full


_NC_CACHE = None


def kernel(**inputs):
    global _NC_CACHE
    from concourse.bass_utils import run_bass_kernel_spmd

    if _NC_CACHE is None:
        _NC_CACHE = build_program()
    res = run_bass_kernel_spmd(_NC_CACHE, make_in_maps(inputs),
                               core_ids=list(range(NCORES)))
    return assemble(res.results)
